# revision 5
# baseline (speedup 1.0000x reference)
"""Trainium2 Bass kernel for nn_Block_9199819948105 (dense_cnn).

Pipeline per core (2 of 16 batches, data-parallel over 8 cores):
  conv1 (stride-2 7^3) as z-Toeplitz banded matmuls accumulating over the
  49 (kx,ky) taps; tensor-product + conv2 via the rank-3 basis decomposition
  (per-(u) grouped convs with basis kernels shared across u -> u rides the
  matmul free dim); 1x1 mix with the learned W2a/W2b; batch-norm stats
  all-reduced across the 8 cores; scale/shift + bias + relu on device.

All weight preprocessing (kernel einsums, banded Toeplitz lhsT construction,
layout packing, bf16 casts) happens host-side in numpy inside kernel().
"""
import sys
import numpy as np

sys.path.insert(0, '/opt/trn_rl_repo')

import ml_dtypes

BF16 = ml_dtypes.bfloat16

# ---------------- problem constants ----------------
N_CORES = 8
B, CIN, D0 = 16, 4, 64
VEC, SOUT, K, NB = 8, 16, 7, 3
D1 = 34          # conv1 output spatial
D2 = 19          # conv2 output spatial
XY1 = D1 * D1    # 1156
XY2 = D2 * D2    # 361
NV2 = D2 * XY2   # 6859
EPS = 1e-5
BB = B // N_CORES
NTOT = B * NV2   # batchnorm element count per channel

# conv1 z-blocking: (zb, win_lo, win_hi, Zo); window = input zi range (clipped)
ZBLK = [(0, 0, 10, 5), (1, 5, 20, 5), (2, 15, 30, 5), (3, 25, 40, 5),
        (4, 35, 50, 5), (5, 45, 60, 5), (6, 55, 64, 4)]
XCH = [(0, 10), (10, 20), (20, 30), (30, 34)]  # conv1 xo chunks (psum banks)


def _xr(k, lo, hi, din):
    """Valid output range [xs, xe) subject to 0 <= 2*xo + k - 5 < din."""
    xs = max(lo, -((k - 5) // 2) if k < 5 else 0)
    # smallest xo with 2*xo + k - 5 >= 0  ->  xo >= (5-k)/2
    xs = max(lo, (5 - k + 1) // 2)
    # largest xo with 2*xo + k - 5 <= din-1 -> xo <= (din + 4 - k)/2
    xe = min(hi, (din + 4 - k) // 2 + 1)
    return xs, xe


# ---------------- host-side weight prep ----------------

def _build_w1t(W1, basis1):
    K1 = np.einsum('uvb,bixyz->uivxyz', W1, basis1[:, :, 0]).reshape(24, 4, K, K, K)
    out = np.zeros((3, 49, 64, 128), np.float32)   # [variant, tap, rows, cols]
    for vi, (nzr, Zo, kzoff) in enumerate([(10, 5, 5), (15, 5, 0), (9, 4, 0)]):
        zr = np.arange(nzr)[:, None]
        zor = np.arange(Zo)[None, :]
        kz = zr - 2 * zor + kzoff                   # [nzr, Zo]
        mask = (kz >= 0) & (kz < 7)
        kzc = np.clip(kz, 0, 6)
        for t in range(49):
            kx, ky = divmod(t, 7)
            # K1[co, ci, kx, ky, kzc] -> [co, ci, nzr, Zo]
            vals = K1[:, :, kx, ky, :][:, :, kzc] * mask  # [24, 4, nzr, Zo]
            # row = 4*zr + ci, col = co*Zo + zor
            m = vals.transpose(2, 1, 0, 3).reshape(4 * nzr, 24 * Zo)
            out[vi, t, :4 * nzr, :24 * Zo] = m
    return out.reshape(3 * 49 * 64, 128).reshape(147, 64, 128)


def _build_w2t(basis2a, basis2b):
    zeta = np.arange(D1)[:, None]
    zo2 = np.arange(D2)[None, :]
    kz = zeta - 2 * zo2 + 5
    mask = (kz >= 0) & (kz < 7)
    kzc = np.clip(kz, 0, 6)
    W = np.zeros((3, 49, 128, 64), np.float32)
    for fam in range(3):
        for t in range(49):
            kx, ky = divmod(t, 7)
            for i in range(3):
                if fam == 0:
                    prof = basis2a[:, 0, i, kx, ky, :]            # [NB, 7]
                elif fam == 1:
                    prof = basis2b[:, 0, i * 3 + i, kx, ky, :]
                else:
                    p = (i + 1) % 3
                    prof = basis2b[:, 0, i * 3 + p, kx, ky, :] + \
                        basis2b[:, 0, p * 3 + i, kx, ky, :]
                for b in range(NB):
                    vals = prof[b][kzc] * mask                    # [D1, D2]
                    W[fam, t, i * D1:(i + 1) * D1, b * D2:(b + 1) * D2] = vals
    return W.reshape(147, 128, 64)


def _build_wmix(W2a, W2b):
    M = np.zeros((48, 16), np.float32)
    for famM, W2 in [(0, W2a), (1, W2b)]:
        for u in range(VEC):
            for b in range(NB):
                M[famM * 24 + u * 3 + b, :] = W2[:, u, b]
    return M


def _prep_s(s_core):
    """[BB,4,64,64,64] -> 7 arrays [BB, 4*win, 64*74] bf16 (row=4*(zi-wlo)+ci),
    free = x*74 + (y+5)  (y padded by 5 both sides)."""
    sp = np.zeros(s_core.shape[:2] + (74, 74, 64), np.float32)
    sp[:, :, 5:69, 5:69, :] = s_core
    out = []
    for zb, wlo, whi, Zo in ZBLK:
        sl = sp[:, :, :, :, wlo:whi]
        sl = np.transpose(sl, (0, 4, 1, 2, 3)).reshape(BB, (whi - wlo) * 4, 74 * 74)
        out.append(np.ascontiguousarray(sl).astype(BF16))
    return out


# ---------------- device program ----------------

def _build_program(n_cores):
    import concourse.bacc as bacc
    import concourse.mybir as mybir
    import concourse.tile as tile

    F32 = mybir.dt.float32
    BF = mybir.dt.bfloat16
    AF = mybir.ActivationFunctionType

    nc = bacc.Bacc("TRN2", target_bir_lowering=False, debug=False,
                   enable_asserts=True, num_devices=n_cores)

    sq_d = [nc.dram_tensor(f"sq{zb}", [BB, (whi - wlo) * 4, 74 * 74], BF,
                           kind="ExternalInput").ap()
            for zb, wlo, whi, Zo in ZBLK]
    w1t_d = nc.dram_tensor("w1t", [64, 147 * 128], BF, kind="ExternalInput").ap()
    w2t_d = nc.dram_tensor("w2t", [128, 147 * 64], BF, kind="ExternalInput").ap()
    wmix_d = nc.dram_tensor("wmix", [48, 16], BF, kind="ExternalInput").ap()
    gvec_d = nc.dram_tensor("gvec", [16, 2], F32, kind="ExternalInput").ap()
    yout_d = nc.dram_tensor("yout", [BB * 16, NV2], F32, kind="ExternalOutput").ap()

    with tile.TileContext(nc) as tc:
        with tc.tile_pool(name="wpool", bufs=1) as wpool, \
             tc.tile_pool(name="big", bufs=1) as big, \
             tc.tile_pool(name="sqp", bufs=1) as sqp, \
             tc.tile_pool(name="vstg", bufs=2) as vstgp, \
             tc.tile_pool(name="tp", bufs=2) as tpp, \
             tc.tile_pool(name="d2s", bufs=2) as d2sp, \
             tc.tile_pool(name="bn", bufs=1) as bnp, \
             tc.tile_pool(name="fz", bufs=1) as fzp, \
             tc.tile_pool(name="ps", bufs=1, space="PSUM") as psp, \
             tc.tile_pool(name="dram", bufs=1, space="DRAM") as dramp:

            w1t = wpool.tile([64, 147 * 128], BF, tag="w1t")
            w2t = wpool.tile([128, 147 * 64], BF, tag="w2t")
            wmix = wpool.tile([48, 16], BF, tag="wmix")
            gvec = wpool.tile([16, 2], F32, tag="gvec")
            nc.sync.dma_start(w1t[:], w1t_d[:])
            nc.sync.dma_start(w2t[:], w2t_d[:])
            nc.sync.dma_start(wmix[:], wmix_d[:])
            nc.sync.dma_start(gvec[:], gvec_d[:])

            FP1 = 44 * 44                       # padded per-u plane ((x+5)*44 + y+5)
            v_main = big.tile([102, VEC * FP1], BF, tag="vmain")
            v_perm = big.tile([102, VEC * FP1], BF, tag="vperm")
            nc.gpsimd.memset(v_main[:], 0.0)
            nc.gpsimd.memset(v_perm[:], 0.0)
            m_in = big.tile([48, NV2], BF, tag="min")
            s1c = bnp.tile([16, 32], F32, tag="s1c")
            s2c = bnp.tile([16, 32], F32, tag="s2c")

            d2_dram = dramp.tile([16, 57 * XY2], BF, tag="d2d")
            ypre_dram = dramp.tile([BB * 16, NV2], F32, tag="ypred")
            bn_in = dramp.tile([16, 2], F32, tag="bnin")
            bn_out = dramp.tile([16, 2], F32, tag="bnout")

            d2v3 = d2_dram[:].rearrange("f (b z) -> f b z", b=3)   # [16, 3, 6859]

            for bb in range(BB):
                # ---------------- conv1 ----------------
                for zb, wlo, whi, Zo in ZBLK:
                    rows = 4 * (whi - wlo)
                    vi = 0 if zb == 0 else (2 if zb == 6 else 1)
                    sqt = sqp.tile([rows, 74 * 74], BF, tag="sqz")
                    eng = (nc.sync, nc.scalar, nc.gpsimd)[zb % 3]
                    eng.dma_start(sqt[:], sq_d[zb][bb])
                    pc = psp.tile([128, 2048], F32, tag="pc")
                    sqv = sqt[0:rows, :].rearrange("p (x y) -> p x y", y=74)
                    for t in range(49):
                        kx, ky = divmod(t, 7)
                        # clip yo to the input-valid range: padded-region
                        # contributions are exactly zero, so skipping those
                        # columns is exact (pending-zero covers first-touch)
                        ys = max(0, (5 - ky + 1) // 2)
                        ye = min(34, (68 - ky) // 2 + 1)
                        ny = ye - ys
                        lhs = w1t[0:rows, (vi * 49 + t) * 128:(vi * 49 + t + 1) * 128]
                        for cc, (clo, chi) in enumerate(XCH):
                            cx = chi - clo
                            xi0 = 2 * clo + kx
                            py0 = 2 * ys + ky
                            rhs = sqv[:, xi0:xi0 + 2 * cx - 1:2,
                                      py0:py0 + 2 * ny - 1:2]
                            outp = (pc[:, cc * 512:cc * 512 + cx * 34]
                                    .rearrange("m (x y) -> m x y", y=34)
                                    [:, :, ys:ye])
                            nc.tensor.matmul(outp, lhs, rhs,
                                             start=(t == 0), stop=(t == 48),
                                             skip_group_check=True)
                    # evac + gather
                    vstg = vstgp.tile([128, XY1], BF, tag="vstg")
                    for cc, (clo, chi) in enumerate(XCH):
                        nch = (chi - clo) * 34
                        nc.vector.tensor_copy(vstg[:, clo * 34:clo * 34 + nch],
                                              pc[:, cc * 512:cc * 512 + nch])
                    vm5 = v_main[:].rearrange("p (u x y) -> p u x y", u=VEC, y=44)
                    for u in range(VEC):
                        for i in range(3):
                            co = 3 * u + i
                            nc.sync.dma_start(
                                vm5[i * D1 + 5 * zb: i * D1 + 5 * zb + Zo, u, 5:39, 5:39],
                                vstg[co * Zo: co * Zo + Zo, :]
                                .rearrange("p (x y) -> p x y", y=34))

                # v_perm rows c*34+z <- v_main rows ((c+1)%3)*34+z
                for c in range(3):
                    p = (c + 1) % 3
                    nc.sync.dma_start(v_perm[c * D1:(c + 1) * D1, :],
                                      v_main[p * D1:(p + 1) * D1, :])

                # ---------------- tensor product + conv2 (d2) ----------------
                vm4 = v_main[:].rearrange("p (u f) -> p u f", u=VEC)
                vp4 = v_perm[:].rearrange("p (u f) -> p u f", u=VEC)
                for u in range(VEC):
                    vmu = vm4[:, u, :]
                    vpu = vp4[:, u, :]
                    t1u = tpp.tile([102, FP1], BF, tag="t1u")
                    t2u = tpp.tile([102, FP1], BF, tag="t2u")
                    nc.vector.tensor_mul(t1u[:], vmu, vmu)
                    nc.vector.tensor_mul(t2u[:], vmu, vpu)
                    pd2a = psp.tile([64, 512], F32, tag="pd2a")
                    pd2b = psp.tile([64, 512], F32, tag="pd2b")
                    pav = pd2a[0:57, 0:XY2].rearrange("p (x y) -> p x y", y=D2)
                    pbv = pd2b[0:57, 0:XY2].rearrange("p (x y) -> p x y", y=D2)
                    for fam, rhs_full, pv in ((0, vmu, pav), (1, t1u[:], pbv),
                                              (2, t2u[:], pbv)):
                        rv = rhs_full.rearrange("p (x y) -> p x y", y=44)
                        for t in range(49):
                            kx, ky = divmod(t, 7)
                            # clip (xo, yo) to the D1=34-valid input window
                            xs = max(0, (5 - kx + 1) // 2)
                            xe = min(19, (38 - kx) // 2 + 1)
                            ys = max(0, (5 - ky + 1) // 2)
                            ye = min(19, (38 - ky) // 2 + 1)
                            rhs = rv[:, kx + 2 * xs:kx + 2 * xs + 2 * (xe - xs) - 1:2,
                                     ky + 2 * ys:ky + 2 * ys + 2 * (ye - ys) - 1:2]
                            lhs = w2t[0:102, (fam * 49 + t) * 64:(fam * 49 + t) * 64 + 57]
                            nc.tensor.matmul(pv[:, xs:xe, ys:ye], lhs, rhs,
                                             start=(t == 0 and fam != 2),
                                             stop=(t == 48 and fam != 1),
                                             skip_group_check=True)
                    for famM, psrc in ((0, pd2a), (1, pd2b)):
                        stg = d2sp.tile([57, XY2], BF, tag=f"stg{famM}")
                        nc.vector.tensor_copy(stg[:], psrc[0:57, 0:XY2])
                        nc.gpsimd.dma_start(d2_dram[famM * 8 + u, :], stg[:])

                # ---------------- mix + stats ----------------
                for famM in range(2):
                    for u in range(VEC):
                        nc.scalar.dma_start(
                            m_in[famM * 24 + u * 3: famM * 24 + u * 3 + 3, :],
                            d2v3[famM * 8 + u])
                nchunks = (NV2 + 511) // 512
                for ch in range(nchunks):
                    c0 = ch * 512
                    cn = min(512, NV2 - c0)
                    pm = psp.tile([16, 512], F32, tag="pm")
                    nc.tensor.matmul(pm[0:16, 0:cn], wmix[:], m_in[:, c0:c0 + cn],
                                     start=True, stop=True)
                    ych = tpp.tile([16, 512], F32, tag="ych")
                    nc.vector.tensor_copy(ych[0:16, 0:cn], pm[0:16, 0:cn])
                    nc.sync.dma_start(ypre_dram[bb * 16:(bb + 1) * 16, c0:c0 + cn],
                                      ych[0:16, 0:cn])
                    nc.vector.reduce_sum(s1c[:, bb * 14 + ch:bb * 14 + ch + 1],
                                         ych[0:16, 0:cn], axis=mybir.AxisListType.X)
                    ysq = tpp.tile([16, 512], F32, tag="ysq")
                    nc.scalar.activation(ysq[0:16, 0:cn], ych[0:16, 0:cn], AF.Square,
                                         accum_out=s2c[:, bb * 14 + ch:bb * 14 + ch + 1])

            # ---------------- batchnorm all-reduce + finalize ----------------
            bnv = bnp.tile([16, 2], F32, tag="bnv")
            nc.vector.reduce_sum(bnv[:, 0:1], s1c[:, 0:28], axis=mybir.AxisListType.X)
            nc.vector.reduce_sum(bnv[:, 1:2], s2c[:, 0:28], axis=mybir.AxisListType.X)
            nc.sync.dma_start(bn_in[:], bnv[:])
            nc.gpsimd.collective_compute(
                "AllReduce", mybir.AluOpType.add,
                replica_groups=[list(range(n_cores))],
                ins=[bn_in[:].opt()], outs=[bn_out[:].opt()])
            # prefetch batch-0 ypre chunks on the sync queue; they overlap
            # the collective because the bn_out read below sits on scalar.
            # Batch-1 reuses the same tiles (loads overlap batch-0 apply).
            nchunks = (NV2 + 511) // 512
            ychs = {}
            for ch in range(nchunks):
                c0 = ch * 512
                cn = min(512, NV2 - c0)
                yc = fzp.tile([16, 512], F32, tag=f"yc{ch}", name=f"yc{ch}")
                nc.sync.dma_start(yc[0:16, 0:cn],
                                  ypre_dram[0:16, c0:c0 + cn])
                ychs[ch] = yc
            bnr = bnp.tile([16, 2], F32, tag="bnr")
            nc.scalar.dma_start(bnr[:], bn_out[:])
            w = bnp.tile([16, 8], F32, tag="bnw")
            invN = 1.0 / float(NTOT)
            nc.vector.tensor_scalar_mul(w[:, 0:1], bnr[:, 0:1], invN)   # mean
            nc.vector.tensor_scalar_mul(w[:, 1:2], bnr[:, 1:2], invN)   # E[x^2]
            nc.vector.tensor_mul(w[:, 2:3], w[:, 0:1], w[:, 0:1])       # mean^2
            nc.vector.tensor_sub(w[:, 3:4], w[:, 1:2], w[:, 2:3])       # var
            nc.vector.tensor_scalar_add(w[:, 4:5], w[:, 3:4], EPS)      # var+eps
            nc.vector.reciprocal(w[:, 5:6], w[:, 4:5])                  # 1/(var+eps)
            nc.scalar.sqrt(w[:, 6:7], w[:, 5:6])                        # rstd
            sc = bnp.tile([16, 2], F32, tag="bnsc")
            nc.vector.tensor_mul(sc[:, 0:1], gvec[:, 0:1], w[:, 6:7])   # scale
            nc.vector.tensor_mul(w[:, 7:8], w[:, 0:1], sc[:, 0:1])      # mean*scale
            nc.vector.tensor_sub(sc[:, 1:2], gvec[:, 1:2], w[:, 7:8])   # shift
            for bb in range(BB):
                for ch in range(nchunks):
                    c0 = ch * 512
                    cn = min(512, NV2 - c0)
                    ych = ychs[ch]
                    if bb == 1:
                        nc.sync.dma_start(ych[0:16, 0:cn],
                                          ypre_dram[16:32, c0:c0 + cn])
                    nc.scalar.activation(ych[0:16, 0:cn], ych[0:16, 0:cn],
                                         AF.Relu,
                                         bias=sc[:, 1:2], scale=sc[:, 0:1])
                    nc.sync.dma_start(yout_d[bb * 16:(bb + 1) * 16, c0:c0 + cn],
                                      ych[0:16, 0:cn])

    nc.compile()
    return nc


_CACHE = {}


def _get_program(n_cores):
    if n_cores not in _CACHE:
        _CACHE[n_cores] = _build_program(n_cores)
    return _CACHE[n_cores]


def _make_in_maps(inputs):
    s = np.asarray(inputs['s'], np.float32)
    w1t = _build_w1t(np.asarray(inputs['W1'], np.float32),
                     np.asarray(inputs['basis1'], np.float32))
    w2t = _build_w2t(np.asarray(inputs['basis2a'], np.float32),
                     np.asarray(inputs['basis2b'], np.float32))
    wmix = _build_wmix(np.asarray(inputs['W2a'], np.float32),
                       np.asarray(inputs['W2b'], np.float32))
    gvec = np.stack([np.asarray(inputs['gamma'], np.float32),
                     np.asarray(inputs['beta'], np.float32)
                     + np.asarray(inputs['bias'], np.float32)], axis=1)
    w1t_b = np.ascontiguousarray(
        w1t.transpose(1, 0, 2).reshape(64, 147 * 128)).astype(BF16)
    w2t_b = np.ascontiguousarray(
        w2t.transpose(1, 0, 2).reshape(128, 147 * 64)).astype(BF16)
    wmix_b = wmix.astype(BF16)
    in_maps = []
    for c in range(N_CORES):
        sqs = _prep_s(s[BB * c: BB * (c + 1)])
        m = {f"sq{zb}": sqs[zb] for zb in range(7)}
        m.update({"w1t": w1t_b, "w2t": w2t_b, "wmix": wmix_b,
                  "gvec": np.ascontiguousarray(gvec)})
        in_maps.append(m)
    return in_maps


def _assemble(results):
    out = np.zeros((B, 16, D2, D2, D2), np.float32)
    for c in range(N_CORES):
        yo = results[c]["yout"]           # [32, 6859]
        for bb in range(BB):
            yb = yo[bb * 16:(bb + 1) * 16].reshape(16, D2, D2, D2)
            out[BB * c + bb] = yb.transpose(0, 2, 3, 1)  # (z,x,y)->(x,y,z)
    return out


def _run(inputs, trace=False, trace_kwargs=None):
    from concourse import bass_utils
    nc = _get_program(N_CORES)
    in_maps = _make_in_maps(inputs)
    res = bass_utils.run_bass_kernel_spmd(
        nc, in_maps, core_ids=list(range(N_CORES)), trace=trace,
        **(trace_kwargs or {}))
    return _assemble(res.results), res


def kernel(**inputs):
    out, _ = _run(inputs, trace=False)
    return out



# revision 6
# speedup vs baseline: 1.0812x; 1.0812x over previous
"""Trainium2 Bass kernel for nn_Block_9199819948105 (dense_cnn).

Pipeline per core (2 of 16 batches, data-parallel over 8 cores):
  conv1 (stride-2 7^3) as z-Toeplitz banded matmuls accumulating over the
  49 (kx,ky) taps; tensor-product + conv2 via the rank-3 basis decomposition
  (per-(u) grouped convs with basis kernels shared across u -> u rides the
  matmul free dim); 1x1 mix with the learned W2a/W2b; batch-norm stats
  all-reduced across the 8 cores; scale/shift + bias + relu on device.

All weight preprocessing (kernel einsums, banded Toeplitz lhsT construction,
layout packing, bf16 casts) happens host-side in numpy inside kernel().
"""
import sys
import numpy as np

sys.path.insert(0, '/opt/trn_rl_repo')

import ml_dtypes

BF16 = ml_dtypes.bfloat16

# ---------------- problem constants ----------------
N_CORES = 8
B, CIN, D0 = 16, 4, 64
VEC, SOUT, K, NB = 8, 16, 7, 3
D1 = 34          # conv1 output spatial
D2 = 19          # conv2 output spatial
XY1 = D1 * D1    # 1156
XY2 = D2 * D2    # 361
NV2 = D2 * XY2   # 6859
EPS = 1e-5
BB = B // N_CORES
NTOT = B * NV2   # batchnorm element count per channel

# conv1 z-blocking: (zb, win_lo, win_hi, Zo); window = input zi range (clipped)
ZBLK = [(0, 0, 10, 5), (1, 5, 20, 5), (2, 15, 30, 5), (3, 25, 40, 5),
        (4, 35, 50, 5), (5, 45, 60, 5), (6, 55, 64, 4)]
XCH = [(0, 10), (10, 20), (20, 30), (30, 34)]  # conv1 xo chunks (psum banks)


def _xr(k, lo, hi, din):
    """Valid output range [xs, xe) subject to 0 <= 2*xo + k - 5 < din."""
    xs = max(lo, -((k - 5) // 2) if k < 5 else 0)
    # smallest xo with 2*xo + k - 5 >= 0  ->  xo >= (5-k)/2
    xs = max(lo, (5 - k + 1) // 2)
    # largest xo with 2*xo + k - 5 <= din-1 -> xo <= (din + 4 - k)/2
    xe = min(hi, (din + 4 - k) // 2 + 1)
    return xs, xe


# ---------------- host-side weight prep ----------------

def _build_w1t(W1, basis1):
    K1 = np.einsum('uvb,bixyz->uivxyz', W1, basis1[:, :, 0]).reshape(24, 4, K, K, K)
    out = np.zeros((3, 49, 64, 128), np.float32)   # [variant, tap, rows, cols]
    for vi, (nzr, Zo, kzoff) in enumerate([(10, 5, 5), (15, 5, 0), (9, 4, 0)]):
        zr = np.arange(nzr)[:, None]
        zor = np.arange(Zo)[None, :]
        kz = zr - 2 * zor + kzoff                   # [nzr, Zo]
        mask = (kz >= 0) & (kz < 7)
        kzc = np.clip(kz, 0, 6)
        for t in range(49):
            kx, ky = divmod(t, 7)
            # K1[co, ci, kx, ky, kzc] -> [co, ci, nzr, Zo]
            vals = K1[:, :, kx, ky, :][:, :, kzc] * mask  # [24, 4, nzr, Zo]
            # row = 4*zr + ci, col = co*Zo + zor
            m = vals.transpose(2, 1, 0, 3).reshape(4 * nzr, 24 * Zo)
            out[vi, t, :4 * nzr, :24 * Zo] = m
    return out.reshape(3 * 49 * 64, 128).reshape(147, 64, 128)


def _build_w2t(basis2a, basis2b):
    zeta = np.arange(D1)[:, None]
    zo2 = np.arange(D2)[None, :]
    kz = zeta - 2 * zo2 + 5
    mask = (kz >= 0) & (kz < 7)
    kzc = np.clip(kz, 0, 6)
    W = np.zeros((3, 49, 128, 64), np.float32)
    for fam in range(3):
        for t in range(49):
            kx, ky = divmod(t, 7)
            for i in range(3):
                if fam == 0:
                    prof = basis2a[:, 0, i, kx, ky, :]            # [NB, 7]
                elif fam == 1:
                    prof = basis2b[:, 0, i * 3 + i, kx, ky, :]
                else:
                    p = (i + 1) % 3
                    prof = basis2b[:, 0, i * 3 + p, kx, ky, :] + \
                        basis2b[:, 0, p * 3 + i, kx, ky, :]
                for b in range(NB):
                    vals = prof[b][kzc] * mask                    # [D1, D2]
                    W[fam, t, i * D1:(i + 1) * D1, b * D2:(b + 1) * D2] = vals
    return W.reshape(147, 128, 64)


def _build_wmix(W2a, W2b):
    M = np.zeros((48, 16), np.float32)
    for famM, W2 in [(0, W2a), (1, W2b)]:
        for u in range(VEC):
            for b in range(NB):
                M[famM * 24 + u * 3 + b, :] = W2[:, u, b]
    return M


def _prep_s(s_core):
    """[BB,4,64,64,64] -> 7 arrays [BB, 4*win, 64*74] bf16 (row=4*(zi-wlo)+ci),
    free = x*74 + (y+5)  (y padded by 5 both sides)."""
    sp = np.zeros(s_core.shape[:2] + (74, 74, 64), np.float32)
    sp[:, :, 5:69, 5:69, :] = s_core
    out = []
    for zb, wlo, whi, Zo in ZBLK:
        sl = sp[:, :, :, :, wlo:whi]
        sl = np.transpose(sl, (0, 4, 1, 2, 3)).reshape(BB, (whi - wlo) * 4, 74 * 74)
        out.append(np.ascontiguousarray(sl).astype(BF16))
    return out


# ---------------- device program ----------------

def _build_program(n_cores):
    import concourse.bacc as bacc
    import concourse.mybir as mybir
    import concourse.tile as tile

    F32 = mybir.dt.float32
    BF = mybir.dt.bfloat16
    AF = mybir.ActivationFunctionType

    nc = bacc.Bacc("TRN2", target_bir_lowering=False, debug=False,
                   enable_asserts=True, num_devices=n_cores)

    sq_d = [nc.dram_tensor(f"sq{zb}", [BB, (whi - wlo) * 4, 74 * 74], BF,
                           kind="ExternalInput").ap()
            for zb, wlo, whi, Zo in ZBLK]
    w1t_d = nc.dram_tensor("w1t", [64, 147 * 128], BF, kind="ExternalInput").ap()
    w2t_d = nc.dram_tensor("w2t", [128, 147 * 64], BF, kind="ExternalInput").ap()
    wmix_d = nc.dram_tensor("wmix", [48, 16], BF, kind="ExternalInput").ap()
    gvec_d = nc.dram_tensor("gvec", [16, 2], F32, kind="ExternalInput").ap()
    yout_d = nc.dram_tensor("yout", [BB * 16, NV2], F32, kind="ExternalOutput").ap()

    with tile.TileContext(nc) as tc:
        with tc.tile_pool(name="wpool", bufs=1) as wpool, \
             tc.tile_pool(name="big", bufs=1) as big, \
             tc.tile_pool(name="sqp", bufs=1) as sqp, \
             tc.tile_pool(name="vstg", bufs=2) as vstgp, \
             tc.tile_pool(name="tp", bufs=2) as tpp, \
             tc.tile_pool(name="d2s", bufs=2) as d2sp, \
             tc.tile_pool(name="bn", bufs=1) as bnp, \
             tc.tile_pool(name="fz", bufs=1) as fzp, \
             tc.tile_pool(name="ps", bufs=1, space="PSUM") as psp, \
             tc.tile_pool(name="dram", bufs=1, space="DRAM") as dramp:

            w1t = wpool.tile([64, 147 * 128], BF, tag="w1t")
            w2t = wpool.tile([128, 147 * 64], BF, tag="w2t")
            wmix = wpool.tile([48, 16], BF, tag="wmix")
            gvec = wpool.tile([16, 2], F32, tag="gvec")
            nc.sync.dma_start(w1t[:], w1t_d[:])
            nc.sync.dma_start(w2t[:], w2t_d[:])
            nc.sync.dma_start(wmix[:], wmix_d[:])
            nc.sync.dma_start(gvec[:], gvec_d[:])

            FP1 = 44 * 44                       # padded per-u plane ((x+5)*44 + y+5)
            v_main = big.tile([102, VEC * FP1], BF, tag="vmain")
            v_perm = big.tile([102, VEC * FP1], BF, tag="vperm")
            nc.gpsimd.memset(v_main[:], 0.0)
            nc.gpsimd.memset(v_perm[:], 0.0)
            m_in = big.tile([48, NV2], BF, tag="min")
            s1c = bnp.tile([16, 32], F32, tag="s1c")
            s2c = bnp.tile([16, 32], F32, tag="s2c")

            d2_dram = dramp.tile([16, 57 * XY2], BF, tag="d2d")
            ypre_dram = dramp.tile([BB * 16, NV2], F32, tag="ypred")
            bn_in = dramp.tile([16, 2], F32, tag="bnin")
            bn_out = dramp.tile([16, 2], F32, tag="bnout")

            d2v3 = d2_dram[:].rearrange("f (b z) -> f b z", b=3)   # [16, 3, 6859]

            for bb in range(BB):
                # ---------------- conv1 ----------------
                for zb, wlo, whi, Zo in ZBLK:
                    rows = 4 * (whi - wlo)
                    vi = 0 if zb == 0 else (2 if zb == 6 else 1)
                    sqt = sqp.tile([rows, 74 * 74], BF, tag="sqz")
                    eng = (nc.sync, nc.scalar, nc.gpsimd)[zb % 3]
                    eng.dma_start(sqt[:], sq_d[zb][bb])
                    pc = psp.tile([128, 2048], F32, tag="pc")
                    sqv = sqt[0:rows, :].rearrange("p (x y) -> p x y", y=74)
                    for t in range(49):
                        kx, ky = divmod(t, 7)
                        lhs = w1t[0:rows, (vi * 49 + t) * 128:(vi * 49 + t + 1) * 128]
                        for cc, (clo, chi) in enumerate(XCH):
                            cx = chi - clo
                            xi0 = 2 * clo + kx
                            rhs = sqv[:, xi0:xi0 + 2 * cx - 1:2, ky:ky + 67:2]
                            outp = pc[:, cc * 512:cc * 512 + cx * 34]
                            nc.tensor.matmul(outp, lhs, rhs,
                                             start=(t == 0), stop=(t == 48))
                    # evac + gather
                    vstg = vstgp.tile([128, XY1], BF, tag="vstg")
                    for cc, (clo, chi) in enumerate(XCH):
                        nch = (chi - clo) * 34
                        nc.vector.tensor_copy(vstg[:, clo * 34:clo * 34 + nch],
                                              pc[:, cc * 512:cc * 512 + nch])
                    vm5 = v_main[:].rearrange("p (u x y) -> p u x y", u=VEC, y=44)
                    for u in range(VEC):
                        for i in range(3):
                            co = 3 * u + i
                            nc.sync.dma_start(
                                vm5[i * D1 + 5 * zb: i * D1 + 5 * zb + Zo, u, 5:39, 5:39],
                                vstg[co * Zo: co * Zo + Zo, :]
                                .rearrange("p (x y) -> p x y", y=34))

                # v_perm rows c*34+z <- v_main rows ((c+1)%3)*34+z
                for c in range(3):
                    p = (c + 1) % 3
                    nc.sync.dma_start(v_perm[c * D1:(c + 1) * D1, :],
                                      v_main[p * D1:(p + 1) * D1, :])

                # ---------------- tensor product + conv2 (d2) ----------------
                vm4 = v_main[:].rearrange("p (u f) -> p u f", u=VEC)
                vp4 = v_perm[:].rearrange("p (u f) -> p u f", u=VEC)
                for u in range(VEC):
                    vmu = vm4[:, u, :]
                    vpu = vp4[:, u, :]
                    t1u = tpp.tile([102, FP1], BF, tag="t1u")
                    t2u = tpp.tile([102, FP1], BF, tag="t2u")
                    nc.vector.tensor_mul(t1u[:], vmu, vmu)
                    nc.vector.tensor_mul(t2u[:], vmu, vpu)
                    pd2a = psp.tile([64, 512], F32, tag="pd2a")
                    pd2b = psp.tile([64, 512], F32, tag="pd2b")
                    pav = pd2a[0:57, 0:XY2].rearrange("p (x y) -> p x y", y=D2)
                    pbv = pd2b[0:57, 0:XY2].rearrange("p (x y) -> p x y", y=D2)
                    for fam, rhs_full, pv in ((0, vmu, pav), (1, t1u[:], pbv),
                                              (2, t2u[:], pbv)):
                        rv = rhs_full.rearrange("p (x y) -> p x y", y=44)
                        for t in range(49):
                            kx, ky = divmod(t, 7)
                            rhs = rv[:, kx:kx + 37:2, ky:ky + 37:2]
                            lhs = w2t[0:102, (fam * 49 + t) * 64:(fam * 49 + t) * 64 + 57]
                            nc.tensor.matmul(pv[:, :, :], lhs, rhs,
                                             start=(t == 0 and fam != 2),
                                             stop=(t == 48 and fam != 1))
                    for famM, psrc in ((0, pd2a), (1, pd2b)):
                        stg = d2sp.tile([57, XY2], BF, tag=f"stg{famM}")
                        nc.vector.tensor_copy(stg[:], psrc[0:57, 0:XY2])
                        nc.gpsimd.dma_start(d2_dram[famM * 8 + u, :], stg[:])

                # ---------------- mix + stats ----------------
                for famM in range(2):
                    for u in range(VEC):
                        nc.scalar.dma_start(
                            m_in[famM * 24 + u * 3: famM * 24 + u * 3 + 3, :],
                            d2v3[famM * 8 + u])
                nchunks = (NV2 + 511) // 512
                for ch in range(nchunks):
                    c0 = ch * 512
                    cn = min(512, NV2 - c0)
                    pm = psp.tile([16, 512], F32, tag="pm")
                    nc.tensor.matmul(pm[0:16, 0:cn], wmix[:], m_in[:, c0:c0 + cn],
                                     start=True, stop=True)
                    ych = tpp.tile([16, 512], F32, tag="ych")
                    nc.vector.tensor_copy(ych[0:16, 0:cn], pm[0:16, 0:cn])
                    nc.sync.dma_start(ypre_dram[bb * 16:(bb + 1) * 16, c0:c0 + cn],
                                      ych[0:16, 0:cn])
                    nc.vector.reduce_sum(s1c[:, bb * 14 + ch:bb * 14 + ch + 1],
                                         ych[0:16, 0:cn], axis=mybir.AxisListType.X)
                    ysq = tpp.tile([16, 512], F32, tag="ysq")
                    nc.scalar.activation(ysq[0:16, 0:cn], ych[0:16, 0:cn], AF.Square,
                                         accum_out=s2c[:, bb * 14 + ch:bb * 14 + ch + 1])

            # ---------------- batchnorm all-reduce + finalize ----------------
            bnv = bnp.tile([16, 2], F32, tag="bnv")
            nc.vector.reduce_sum(bnv[:, 0:1], s1c[:, 0:28], axis=mybir.AxisListType.X)
            nc.vector.reduce_sum(bnv[:, 1:2], s2c[:, 0:28], axis=mybir.AxisListType.X)
            nc.sync.dma_start(bn_in[:], bnv[:])
            nc.gpsimd.collective_compute(
                "AllReduce", mybir.AluOpType.add,
                replica_groups=[list(range(n_cores))],
                ins=[bn_in[:].opt()], outs=[bn_out[:].opt()])
            # prefetch batch-0 ypre chunks on the sync queue; they overlap
            # the collective because the bn_out read below sits on scalar.
            # Batch-1 reuses the same tiles (loads overlap batch-0 apply).
            nchunks = (NV2 + 511) // 512
            ychs = {}
            for ch in range(nchunks):
                c0 = ch * 512
                cn = min(512, NV2 - c0)
                yc = fzp.tile([16, 512], F32, tag=f"yc{ch}", name=f"yc{ch}")
                nc.sync.dma_start(yc[0:16, 0:cn],
                                  ypre_dram[0:16, c0:c0 + cn])
                ychs[ch] = yc
            bnr = bnp.tile([16, 2], F32, tag="bnr")
            nc.scalar.dma_start(bnr[:], bn_out[:])
            w = bnp.tile([16, 8], F32, tag="bnw")
            invN = 1.0 / float(NTOT)
            nc.vector.tensor_scalar_mul(w[:, 0:1], bnr[:, 0:1], invN)   # mean
            nc.vector.tensor_scalar_mul(w[:, 1:2], bnr[:, 1:2], invN)   # E[x^2]
            nc.vector.tensor_mul(w[:, 2:3], w[:, 0:1], w[:, 0:1])       # mean^2
            nc.vector.tensor_sub(w[:, 3:4], w[:, 1:2], w[:, 2:3])       # var
            nc.vector.tensor_scalar_add(w[:, 4:5], w[:, 3:4], EPS)      # var+eps
            nc.vector.reciprocal(w[:, 5:6], w[:, 4:5])                  # 1/(var+eps)
            nc.scalar.sqrt(w[:, 6:7], w[:, 5:6])                        # rstd
            sc = bnp.tile([16, 2], F32, tag="bnsc")
            nc.vector.tensor_mul(sc[:, 0:1], gvec[:, 0:1], w[:, 6:7])   # scale
            nc.vector.tensor_mul(w[:, 7:8], w[:, 0:1], sc[:, 0:1])      # mean*scale
            nc.vector.tensor_sub(sc[:, 1:2], gvec[:, 1:2], w[:, 7:8])   # shift
            for bb in range(BB):
                for ch in range(nchunks):
                    c0 = ch * 512
                    cn = min(512, NV2 - c0)
                    ych = ychs[ch]
                    if bb == 1:
                        nc.sync.dma_start(ych[0:16, 0:cn],
                                          ypre_dram[16:32, c0:c0 + cn])
                    nc.scalar.activation(ych[0:16, 0:cn], ych[0:16, 0:cn],
                                         AF.Relu,
                                         bias=sc[:, 1:2], scale=sc[:, 0:1])
                    nc.sync.dma_start(yout_d[bb * 16:(bb + 1) * 16, c0:c0 + cn],
                                      ych[0:16, 0:cn])

    nc.compile()
    return nc


_CACHE = {}


def _get_program(n_cores):
    if n_cores not in _CACHE:
        _CACHE[n_cores] = _build_program(n_cores)
    return _CACHE[n_cores]


def _make_in_maps(inputs):
    s = np.asarray(inputs['s'], np.float32)
    w1t = _build_w1t(np.asarray(inputs['W1'], np.float32),
                     np.asarray(inputs['basis1'], np.float32))
    w2t = _build_w2t(np.asarray(inputs['basis2a'], np.float32),
                     np.asarray(inputs['basis2b'], np.float32))
    wmix = _build_wmix(np.asarray(inputs['W2a'], np.float32),
                       np.asarray(inputs['W2b'], np.float32))
    gvec = np.stack([np.asarray(inputs['gamma'], np.float32),
                     np.asarray(inputs['beta'], np.float32)
                     + np.asarray(inputs['bias'], np.float32)], axis=1)
    w1t_b = np.ascontiguousarray(
        w1t.transpose(1, 0, 2).reshape(64, 147 * 128)).astype(BF16)
    w2t_b = np.ascontiguousarray(
        w2t.transpose(1, 0, 2).reshape(128, 147 * 64)).astype(BF16)
    wmix_b = wmix.astype(BF16)
    in_maps = []
    for c in range(N_CORES):
        sqs = _prep_s(s[BB * c: BB * (c + 1)])
        m = {f"sq{zb}": sqs[zb] for zb in range(7)}
        m.update({"w1t": w1t_b, "w2t": w2t_b, "wmix": wmix_b,
                  "gvec": np.ascontiguousarray(gvec)})
        in_maps.append(m)
    return in_maps


def _assemble(results):
    out = np.zeros((B, 16, D2, D2, D2), np.float32)
    for c in range(N_CORES):
        yo = results[c]["yout"]           # [32, 6859]
        for bb in range(BB):
            yb = yo[bb * 16:(bb + 1) * 16].reshape(16, D2, D2, D2)
            out[BB * c + bb] = yb.transpose(0, 2, 3, 1)  # (z,x,y)->(x,y,z)
    return out


def _run(inputs, trace=False, trace_kwargs=None):
    from concourse import bass_utils
    nc = _get_program(N_CORES)
    in_maps = _make_in_maps(inputs)
    res = bass_utils.run_bass_kernel_spmd(
        nc, in_maps, core_ids=list(range(N_CORES)), trace=trace,
        **(trace_kwargs or {}))
    return _assemble(res.results), res


def kernel(**inputs):
    out, _ = _run(inputs, trace=False)
    return out



# revision 7
# speedup vs baseline: 1.0947x; 1.0125x over previous
"""Trainium2 Bass kernel for nn_Block_9199819948105 (dense_cnn).

Pipeline per core (2 of 16 batches, data-parallel over 8 cores):
  conv1 (stride-2 7^3) as z-Toeplitz banded matmuls accumulating over the
  49 (kx,ky) taps; tensor-product + conv2 via the rank-3 basis decomposition
  (per-(u) grouped convs with basis kernels shared across u -> u rides the
  matmul free dim); 1x1 mix with the learned W2a/W2b; batch-norm stats
  all-reduced across the 8 cores; scale/shift + bias + relu on device.

All weight preprocessing (kernel einsums, banded Toeplitz lhsT construction,
layout packing, bf16 casts) happens host-side in numpy inside kernel().
"""
import sys
import numpy as np

sys.path.insert(0, '/opt/trn_rl_repo')

import ml_dtypes

BF16 = ml_dtypes.bfloat16

# ---------------- problem constants ----------------
N_CORES = 8
B, CIN, D0 = 16, 4, 64
VEC, SOUT, K, NB = 8, 16, 7, 3
D1 = 34          # conv1 output spatial
D2 = 19          # conv2 output spatial
XY1 = D1 * D1    # 1156
XY2 = D2 * D2    # 361
NV2 = D2 * XY2   # 6859
EPS = 1e-5
BB = B // N_CORES
NTOT = B * NV2   # batchnorm element count per channel

# conv1 z-blocking: (zb, win_lo, win_hi, Zo); window = input zi range (clipped)
ZBLK = [(0, 0, 10, 5), (1, 5, 20, 5), (2, 15, 30, 5), (3, 25, 40, 5),
        (4, 35, 50, 5), (5, 45, 60, 5), (6, 55, 64, 4)]
XCH = [(0, 10), (10, 20), (20, 30), (30, 34)]  # conv1 xo chunks (psum banks)


def _xr(k, lo, hi, din):
    """Valid output range [xs, xe) subject to 0 <= 2*xo + k - 5 < din."""
    xs = max(lo, -((k - 5) // 2) if k < 5 else 0)
    # smallest xo with 2*xo + k - 5 >= 0  ->  xo >= (5-k)/2
    xs = max(lo, (5 - k + 1) // 2)
    # largest xo with 2*xo + k - 5 <= din-1 -> xo <= (din + 4 - k)/2
    xe = min(hi, (din + 4 - k) // 2 + 1)
    return xs, xe


# ---------------- host-side weight prep ----------------

def _build_w1t(W1, basis1):
    K1 = np.einsum('uvb,bixyz->uivxyz', W1, basis1[:, :, 0]).reshape(24, 4, K, K, K)
    out = np.zeros((3, 49, 64, 128), np.float32)   # [variant, tap, rows, cols]
    for vi, (nzr, Zo, kzoff) in enumerate([(10, 5, 5), (15, 5, 0), (9, 4, 0)]):
        zr = np.arange(nzr)[:, None]
        zor = np.arange(Zo)[None, :]
        kz = zr - 2 * zor + kzoff                   # [nzr, Zo]
        mask = (kz >= 0) & (kz < 7)
        kzc = np.clip(kz, 0, 6)
        for t in range(49):
            kx, ky = divmod(t, 7)
            # K1[co, ci, kx, ky, kzc] -> [co, ci, nzr, Zo]
            vals = K1[:, :, kx, ky, :][:, :, kzc] * mask  # [24, 4, nzr, Zo]
            # row = 4*zr + ci, col = co*Zo + zor
            m = vals.transpose(2, 1, 0, 3).reshape(4 * nzr, 24 * Zo)
            out[vi, t, :4 * nzr, :24 * Zo] = m
    return out.reshape(3 * 49 * 64, 128).reshape(147, 64, 128)


def _build_w2t(basis2a, basis2b):
    zeta = np.arange(D1)[:, None]
    zo2 = np.arange(D2)[None, :]
    kz = zeta - 2 * zo2 + 5
    mask = (kz >= 0) & (kz < 7)
    kzc = np.clip(kz, 0, 6)
    W = np.zeros((3, 49, 128, 64), np.float32)
    for fam in range(3):
        for t in range(49):
            kx, ky = divmod(t, 7)
            for i in range(3):
                if fam == 0:
                    prof = basis2a[:, 0, i, kx, ky, :]            # [NB, 7]
                elif fam == 1:
                    prof = basis2b[:, 0, i * 3 + i, kx, ky, :]
                else:
                    p = (i + 1) % 3
                    prof = basis2b[:, 0, i * 3 + p, kx, ky, :] + \
                        basis2b[:, 0, p * 3 + i, kx, ky, :]
                for b in range(NB):
                    vals = prof[b][kzc] * mask                    # [D1, D2]
                    W[fam, t, i * D1:(i + 1) * D1, b * D2:(b + 1) * D2] = vals
    return W.reshape(147, 128, 64)


def _build_wmix(W2a, W2b):
    M = np.zeros((48, 16), np.float32)
    for famM, W2 in [(0, W2a), (1, W2b)]:
        for u in range(VEC):
            for b in range(NB):
                M[famM * 24 + u * 3 + b, :] = W2[:, u, b]
    return M


def _prep_s(s_core):
    """[BB,4,64,64,64] -> 7 arrays [BB, 4*win, 64*74] bf16 (row=4*(zi-wlo)+ci),
    free = x*74 + (y+5)  (y padded by 5 both sides)."""
    sp = np.zeros(s_core.shape[:2] + (74, 74, 64), np.float32)
    sp[:, :, 5:69, 5:69, :] = s_core
    out = []
    for zb, wlo, whi, Zo in ZBLK:
        sl = sp[:, :, :, :, wlo:whi]
        sl = np.transpose(sl, (0, 4, 1, 2, 3)).reshape(BB, (whi - wlo) * 4, 74 * 74)
        out.append(np.ascontiguousarray(sl).astype(BF16))
    return out


# ---------------- device program ----------------

def _build_program(n_cores):
    import concourse.bacc as bacc
    import concourse.mybir as mybir
    import concourse.tile as tile

    F32 = mybir.dt.float32
    BF = mybir.dt.bfloat16
    AF = mybir.ActivationFunctionType

    nc = bacc.Bacc("TRN2", target_bir_lowering=False, debug=False,
                   enable_asserts=True, num_devices=n_cores)

    sq_d = [nc.dram_tensor(f"sq{zb}", [BB, (whi - wlo) * 4, 74 * 74], BF,
                           kind="ExternalInput").ap()
            for zb, wlo, whi, Zo in ZBLK]
    w1t_d = nc.dram_tensor("w1t", [64, 147 * 128], BF, kind="ExternalInput").ap()
    w2t_d = nc.dram_tensor("w2t", [128, 147 * 64], BF, kind="ExternalInput").ap()
    wmix_d = nc.dram_tensor("wmix", [48, 16], BF, kind="ExternalInput").ap()
    gvec_d = nc.dram_tensor("gvec", [16, 2], F32, kind="ExternalInput").ap()
    yout_d = nc.dram_tensor("yout", [BB * 16, NV2], F32, kind="ExternalOutput").ap()

    with tile.TileContext(nc) as tc:
        with tc.tile_pool(name="wpool", bufs=1) as wpool, \
             tc.tile_pool(name="big", bufs=1) as big, \
             tc.tile_pool(name="sqp", bufs=1) as sqp, \
             tc.tile_pool(name="vstg", bufs=2) as vstgp, \
             tc.tile_pool(name="tp", bufs=2) as tpp, \
             tc.tile_pool(name="d2s", bufs=2) as d2sp, \
             tc.tile_pool(name="bn", bufs=1) as bnp, \
             tc.tile_pool(name="fz", bufs=1) as fzp, \
             tc.tile_pool(name="ps", bufs=1, space="PSUM") as psp, \
             tc.tile_pool(name="dram", bufs=1, space="DRAM") as dramp:

            w1t = wpool.tile([64, 147 * 128], BF, tag="w1t")
            w2t = wpool.tile([128, 147 * 64], BF, tag="w2t")
            wmix = wpool.tile([48, 16], BF, tag="wmix")
            gvec = wpool.tile([16, 2], F32, tag="gvec")
            nc.sync.dma_start(w1t[:], w1t_d[:])
            nc.sync.dma_start(w2t[:], w2t_d[:])
            nc.sync.dma_start(wmix[:], wmix_d[:])
            nc.sync.dma_start(gvec[:], gvec_d[:])

            FP1 = 44 * 44                       # padded per-u plane ((x+5)*44 + y+5)
            v_main = big.tile([102, VEC * FP1], BF, tag="vmain")
            v_perm = big.tile([102, VEC * FP1], BF, tag="vperm")
            nc.gpsimd.memset(v_main[:], 0.0)
            nc.gpsimd.memset(v_perm[:], 0.0)
            m_in = big.tile([48, NV2], BF, tag="min")
            s1c = bnp.tile([16, 32], F32, tag="s1c")
            s2c = bnp.tile([16, 32], F32, tag="s2c")

            d2_dram = dramp.tile([16, 57 * XY2], BF, tag="d2d")
            ypre_dram = dramp.tile([BB * 16, NV2], F32, tag="ypred")
            bn_in = dramp.tile([16, 2], F32, tag="bnin")
            bn_out = dramp.tile([16, 2], F32, tag="bnout")

            d2v3 = d2_dram[:].rearrange("f (b z) -> f b z", b=3)   # [16, 3, 6859]

            for bb in range(BB):
                # ---------------- conv1 ----------------
                for zb, wlo, whi, Zo in ZBLK:
                    rows = 4 * (whi - wlo)
                    vi = 0 if zb == 0 else (2 if zb == 6 else 1)
                    sqt = sqp.tile([rows, 74 * 74], BF, tag="sqz")
                    eng = (nc.sync, nc.scalar, nc.gpsimd)[zb % 3]
                    eng.dma_start(sqt[:], sq_d[zb][bb])
                    pc = psp.tile([128, 2048], F32, tag="pc")
                    sqv = sqt[0:rows, :].rearrange("p (x y) -> p x y", y=74)
                    for t in range(49):
                        kx, ky = divmod(t, 7)
                        lhs = w1t[0:rows, (vi * 49 + t) * 128:(vi * 49 + t + 1) * 128]
                        for cc, (clo, chi) in enumerate(XCH):
                            cx = chi - clo
                            xi0 = 2 * clo + kx
                            rhs = sqv[:, xi0:xi0 + 2 * cx - 1:2, ky:ky + 67:2]
                            outp = pc[:, cc * 512:cc * 512 + cx * 34]
                            nc.tensor.matmul(outp, lhs, rhs,
                                             start=(t == 0), stop=(t == 48))
                    # evac + gather
                    vstg = vstgp.tile([128, XY1], BF, tag="vstg")
                    for cc, (clo, chi) in enumerate(XCH):
                        nch = (chi - clo) * 34
                        nc.vector.tensor_copy(vstg[:, clo * 34:clo * 34 + nch],
                                              pc[:, cc * 512:cc * 512 + nch])
                    vm5 = v_main[:].rearrange("p (u x y) -> p u x y", u=VEC, y=44)
                    for u in range(VEC):
                        for i in range(3):
                            co = 3 * u + i
                            eng = (nc.sync, nc.scalar, nc.gpsimd)[co % 3]
                            eng.dma_start(
                                vm5[i * D1 + 5 * zb: i * D1 + 5 * zb + Zo, u, 5:39, 5:39],
                                vstg[co * Zo: co * Zo + Zo, :]
                                .rearrange("p (x y) -> p x y", y=34))

                # v_perm rows c*34+z <- v_main rows ((c+1)%3)*34+z
                for c in range(3):
                    p = (c + 1) % 3
                    (nc.scalar, nc.gpsimd, nc.sync)[c].dma_start(
                        v_perm[c * D1:(c + 1) * D1, :],
                        v_main[p * D1:(p + 1) * D1, :])

                # ---------------- tensor product + conv2 (d2) ----------------
                vm4 = v_main[:].rearrange("p (u f) -> p u f", u=VEC)
                vp4 = v_perm[:].rearrange("p (u f) -> p u f", u=VEC)
                for u in range(VEC):
                    vmu = vm4[:, u, :]
                    vpu = vp4[:, u, :]
                    t1u = tpp.tile([102, FP1], BF, tag="t1u")
                    t2u = tpp.tile([102, FP1], BF, tag="t2u")
                    nc.vector.tensor_mul(t1u[:], vmu, vmu)
                    nc.vector.tensor_mul(t2u[:], vmu, vpu)
                    pd2a = psp.tile([64, 512], F32, tag="pd2a")
                    pd2b = psp.tile([64, 512], F32, tag="pd2b")
                    pav = pd2a[0:57, 0:XY2].rearrange("p (x y) -> p x y", y=D2)
                    pbv = pd2b[0:57, 0:XY2].rearrange("p (x y) -> p x y", y=D2)
                    for fam, rhs_full, pv in ((0, vmu, pav), (1, t1u[:], pbv),
                                              (2, t2u[:], pbv)):
                        rv = rhs_full.rearrange("p (x y) -> p x y", y=44)
                        for t in range(49):
                            kx, ky = divmod(t, 7)
                            rhs = rv[:, kx:kx + 37:2, ky:ky + 37:2]
                            lhs = w2t[0:102, (fam * 49 + t) * 64:(fam * 49 + t) * 64 + 57]
                            nc.tensor.matmul(pv[:, :, :], lhs, rhs,
                                             start=(t == 0 and fam != 2),
                                             stop=(t == 48 and fam != 1))
                    for famM, psrc in ((0, pd2a), (1, pd2b)):
                        stg = d2sp.tile([57, XY2], BF, tag=f"stg{famM}")
                        nc.vector.tensor_copy(stg[:], psrc[0:57, 0:XY2])
                        nc.gpsimd.dma_start(d2_dram[famM * 8 + u, :], stg[:])

                # ---------------- mix + stats ----------------
                for famM in range(2):
                    for u in range(VEC):
                        nc.scalar.dma_start(
                            m_in[famM * 24 + u * 3: famM * 24 + u * 3 + 3, :],
                            d2v3[famM * 8 + u])
                nchunks = (NV2 + 511) // 512
                for ch in range(nchunks):
                    c0 = ch * 512
                    cn = min(512, NV2 - c0)
                    pm = psp.tile([16, 512], F32, tag="pm")
                    nc.tensor.matmul(pm[0:16, 0:cn], wmix[:], m_in[:, c0:c0 + cn],
                                     start=True, stop=True)
                    ych = tpp.tile([16, 512], F32, tag="ych")
                    nc.vector.tensor_copy(ych[0:16, 0:cn], pm[0:16, 0:cn])
                    nc.gpsimd.dma_start(ypre_dram[bb * 16:(bb + 1) * 16, c0:c0 + cn],
                                        ych[0:16, 0:cn])
                    nc.vector.reduce_sum(s1c[:, bb * 14 + ch:bb * 14 + ch + 1],
                                         ych[0:16, 0:cn], axis=mybir.AxisListType.X)
                    ysq = tpp.tile([16, 512], F32, tag="ysq")
                    nc.scalar.activation(ysq[0:16, 0:cn], ych[0:16, 0:cn], AF.Square,
                                         accum_out=s2c[:, bb * 14 + ch:bb * 14 + ch + 1])

            # ---------------- batchnorm all-reduce + finalize ----------------
            bnv = bnp.tile([16, 2], F32, tag="bnv")
            nc.vector.reduce_sum(bnv[:, 0:1], s1c[:, 0:28], axis=mybir.AxisListType.X)
            nc.vector.reduce_sum(bnv[:, 1:2], s2c[:, 0:28], axis=mybir.AxisListType.X)
            nc.sync.dma_start(bn_in[:], bnv[:])
            nc.gpsimd.collective_compute(
                "AllReduce", mybir.AluOpType.add,
                replica_groups=[list(range(n_cores))],
                ins=[bn_in[:].opt()], outs=[bn_out[:].opt()])
            # prefetch batch-0 ypre chunks on the sync queue; they overlap
            # the collective because the bn_out read below sits on scalar.
            # Batch-1 reuses the same tiles (loads overlap batch-0 apply).
            nchunks = (NV2 + 511) // 512
            ychs = {}
            for ch in range(nchunks):
                c0 = ch * 512
                cn = min(512, NV2 - c0)
                yc = fzp.tile([16, 512], F32, tag=f"yc{ch}", name=f"yc{ch}")
                nc.sync.dma_start(yc[0:16, 0:cn],
                                  ypre_dram[0:16, c0:c0 + cn])
                ychs[ch] = yc
            bnr = bnp.tile([16, 2], F32, tag="bnr")
            nc.scalar.dma_start(bnr[:], bn_out[:])
            w = bnp.tile([16, 8], F32, tag="bnw")
            invN = 1.0 / float(NTOT)
            nc.vector.tensor_scalar_mul(w[:, 0:1], bnr[:, 0:1], invN)   # mean
            nc.vector.tensor_scalar_mul(w[:, 1:2], bnr[:, 1:2], invN)   # E[x^2]
            nc.vector.tensor_mul(w[:, 2:3], w[:, 0:1], w[:, 0:1])       # mean^2
            nc.vector.tensor_sub(w[:, 3:4], w[:, 1:2], w[:, 2:3])       # var
            nc.vector.tensor_scalar_add(w[:, 4:5], w[:, 3:4], EPS)      # var+eps
            nc.vector.reciprocal(w[:, 5:6], w[:, 4:5])                  # 1/(var+eps)
            nc.scalar.sqrt(w[:, 6:7], w[:, 5:6])                        # rstd
            sc = bnp.tile([16, 2], F32, tag="bnsc")
            nc.vector.tensor_mul(sc[:, 0:1], gvec[:, 0:1], w[:, 6:7])   # scale
            nc.vector.tensor_mul(w[:, 7:8], w[:, 0:1], sc[:, 0:1])      # mean*scale
            nc.vector.tensor_sub(sc[:, 1:2], gvec[:, 1:2], w[:, 7:8])   # shift
            for bb in range(BB):
                for ch in range(nchunks):
                    c0 = ch * 512
                    cn = min(512, NV2 - c0)
                    ych = ychs[ch]
                    if bb == 1:
                        nc.sync.dma_start(ych[0:16, 0:cn],
                                          ypre_dram[16:32, c0:c0 + cn])
                    nc.scalar.activation(ych[0:16, 0:cn], ych[0:16, 0:cn],
                                         AF.Relu,
                                         bias=sc[:, 1:2], scale=sc[:, 0:1])
                    nc.sync.dma_start(yout_d[bb * 16:(bb + 1) * 16, c0:c0 + cn],
                                      ych[0:16, 0:cn])

    nc.compile()
    return nc


_CACHE = {}


def _get_program(n_cores):
    if n_cores not in _CACHE:
        _CACHE[n_cores] = _build_program(n_cores)
    return _CACHE[n_cores]


def _make_in_maps(inputs):
    s = np.asarray(inputs['s'], np.float32)
    w1t = _build_w1t(np.asarray(inputs['W1'], np.float32),
                     np.asarray(inputs['basis1'], np.float32))
    w2t = _build_w2t(np.asarray(inputs['basis2a'], np.float32),
                     np.asarray(inputs['basis2b'], np.float32))
    wmix = _build_wmix(np.asarray(inputs['W2a'], np.float32),
                       np.asarray(inputs['W2b'], np.float32))
    gvec = np.stack([np.asarray(inputs['gamma'], np.float32),
                     np.asarray(inputs['beta'], np.float32)
                     + np.asarray(inputs['bias'], np.float32)], axis=1)
    w1t_b = np.ascontiguousarray(
        w1t.transpose(1, 0, 2).reshape(64, 147 * 128)).astype(BF16)
    w2t_b = np.ascontiguousarray(
        w2t.transpose(1, 0, 2).reshape(128, 147 * 64)).astype(BF16)
    wmix_b = wmix.astype(BF16)
    in_maps = []
    for c in range(N_CORES):
        sqs = _prep_s(s[BB * c: BB * (c + 1)])
        m = {f"sq{zb}": sqs[zb] for zb in range(7)}
        m.update({"w1t": w1t_b, "w2t": w2t_b, "wmix": wmix_b,
                  "gvec": np.ascontiguousarray(gvec)})
        in_maps.append(m)
    return in_maps


def _assemble(results):
    out = np.zeros((B, 16, D2, D2, D2), np.float32)
    for c in range(N_CORES):
        yo = results[c]["yout"]           # [32, 6859]
        for bb in range(BB):
            yb = yo[bb * 16:(bb + 1) * 16].reshape(16, D2, D2, D2)
            out[BB * c + bb] = yb.transpose(0, 2, 3, 1)  # (z,x,y)->(x,y,z)
    return out


def _run(inputs, trace=False, trace_kwargs=None):
    from concourse import bass_utils
    nc = _get_program(N_CORES)
    in_maps = _make_in_maps(inputs)
    res = bass_utils.run_bass_kernel_spmd(
        nc, in_maps, core_ids=list(range(N_CORES)), trace=trace,
        **(trace_kwargs or {}))
    return _assemble(res.results), res


def kernel(**inputs):
    out, _ = _run(inputs, trace=False)
    return out

